# revision 1
# baseline (speedup 1.0000x reference)
"""Trainium2 Bass kernel for masked multi-head attention.

Reference computation (B=4, T=2048, D=1024, H=16, dh=64):
    qp = q @ Wq.T + bq ; kp = k @ Wk.T + bk ; vp = v @ Wv.T + bv
    s  = (qh @ khT) / 8 ; s = where(mask, -1e6, s) ; p = softmax(s)
    o  = p @ vh ; y = o @ Wo.T + bo

Sharding: 8 cores = (batch b in 0..3) x (head-group g in 0..1).
Each core handles batch b and 8 heads (512 channels), computes a partial
y^T (output projection over its 512 channels); host sums core pairs,
transposes, and adds the bias terms.

Per-core device algorithm (everything in transposed "T-major" layouts so
no on-device transposes are needed):
  A) qpT[c,t] = sum_m WqT[m,c] * qT[m,t]   (float32r matmuls, psum acc)
     kpT likewise.  1/8 score scale folded into WqT host-side.
  B) vp[t,c]  = sum_m vT[m,t] * WvT[m,c]   (untransposed; stored bf16 in
     a [t, 8*65] layout with a ones-column per head for row-sums)
  C) per (qcol, head): ST[k,q] = khT.T @ qhT (f32r) -> exp on ACT (bf16)
     -> multiply by maskT tile (DVE, bf16 2x) -> OT[d,q] (+= over k-tiles,
     PE, bf16) with row 64 = sum_k p~[k,q] (ones column).
     Normalize: otn = OT[0:64] * (1/r) broadcast (DVE + gpsimd bcast).
  D) yT[j,t] = sum_c wot[c,j] * otn[c,t]   (bf16) -> DMA out.
"""

import sys
import numpy as np

for _p in ("/opt/trn_rl_repo",):
    if _p not in sys.path:
        sys.path.insert(0, _p)

import ml_dtypes
from contextlib import ExitStack

import concourse.bass as bass
import concourse.tile as tile
from concourse import bacc, mybir
from concourse import bass_utils

B, T, D, H = 4, 2048, 1024, 16
DH = 64          # head dim
HC = 8           # heads per core
C = HC * DH      # 512 channels per core
F32 = mybir.dt.float32
F32R = mybir.dt.float32r
BF16 = mybir.dt.bfloat16
EXP = mybir.ActivationFunctionType.Exp

_CACHED = {}
OT_BUFS = 1
D_MODE = "interleaved"  # or "after"


def _build_nc():
    nc = bacc.Bacc("TRN2", target_bir_lowering=False, debug=False,
                   enable_asserts=False)
    qt = nc.dram_tensor("qt", [D, T], BF16, kind="ExternalInput").ap()
    kt = nc.dram_tensor("kt", [D, T], BF16, kind="ExternalInput").ap()
    vt = nc.dram_tensor("vt", [D, T], BF16, kind="ExternalInput").ap()
    wqt = nc.dram_tensor("wqt", [D, C], BF16, kind="ExternalInput").ap()
    wkt = nc.dram_tensor("wkt", [D, C], BF16, kind="ExternalInput").ap()
    wvt = nc.dram_tensor("wvt", [D, C], BF16, kind="ExternalInput").ap()
    wot = nc.dram_tensor("wot", [C, D], BF16, kind="ExternalInput").ap()
    maskt = nc.dram_tensor("maskt", [T, T], BF16, kind="ExternalInput").ap()
    bqt = nc.dram_tensor("bqt", [C, 1], F32, kind="ExternalInput").ap()
    bkt = nc.dram_tensor("bkt", [C, 1], F32, kind="ExternalInput").ap()
    yt = nc.dram_tensor("yt", [D, T], F32, kind="ExternalOutput").ap()

    with tile.TileContext(nc) as tc, ExitStack() as ctx:
        _emit(ctx, tc, qt, kt, vt, wqt, wkt, wvt, wot, maskt, bqt, bkt, yt)
    nc.compile()
    return nc



def _emit_d_jbs(nc, pypool, ye, wo_all, otn_sb, qc, jbs, altpool=None):
    NCT = C // 128
    for jb in jbs:
        pool = altpool if (altpool is not None and jb % 2) else pypool
        tg = "pv" if (altpool is not None and jb % 2) else "py"
        ps = pool.tile([128, 512], F32, tag=tg, name="psy")
        for ct in range(NCT):
            lhs = wo_all[:, ct * D + jb * 128:ct * D + (jb + 1) * 128]
            rhs = otn_sb[ct][:, qc * 512:(qc + 1) * 512]
            nc.tensor.matmul(ps[:, :], lhs, rhs,
                             start=(ct == 0), stop=(ct == NCT - 1))
        nc.vector.tensor_copy(ye[:, (jb % 2) * 512:(jb % 2 + 1) * 512],
                              ps[:, :])


def _emit_d_store(nc, ye, yt, qc, pair):
    nc.sync.dma_start(
        yt.rearrange("(jb p) t -> p jb t",
                     p=128)[:, pair * 2:(pair + 1) * 2,
                            qc * 512:(qc + 1) * 512],
        ye.rearrange("p (jb t) -> p jb t", jb=2))


def _emit(ctx, tc, qt, kt, vt, wqt, wkt, wvt, wot, maskt, bqt, bkt, yt):
    nc = tc.nc
    NKT = T // 128      # 16 k-tiles
    NQC = 4             # q columns of 512
    NCT = C // 128      # 4 channel tiles (= head pairs)
    NM = D // 128       # 8 contraction tiles

    # ---- persistent SBUF arrays -------------------------------------
    persist = ctx.enter_context(tc.tile_pool(name="persist", bufs=1))
    qpt_sb = [persist.tile([128, T], F32R, tag=f"qpt{i}", name=f"qpt{i}")
              for i in range(NCT)]
    kpt_sb = [persist.tile([128, T], F32R, tag=f"kpt{i}", name=f"kpt{i}")
              for i in range(NCT)]
    VPW = HC * (DH + 1)  # 520
    vp_ext = persist.tile([128, NKT * VPW], BF16, tag="vpext", name="vpext")
    otn_sb = [persist.tile([128, T], BF16, tag=f"otn{i}", name=f"otn{i}")
              for i in range(NCT)]
    bias_sb = persist.tile([128, 2 * NCT], F32, tag="bias", name="bias")
    wo_all = persist.tile([128, NCT * D], BF16, tag="wo", name="wo_all")

    nc.gpsimd.memset(vp_ext[:, :], 1.0)
    nc.sync.dma_start(bias_sb[:, 0:NCT],
                      bqt.rearrange("(c p) o -> p (c o)", p=128))
    nc.sync.dma_start(bias_sb[:, NCT:2 * NCT],
                      bkt.rearrange("(c p) o -> p (c o)", p=128))

    # mask tiles double-buffered; prefetch qc0/qc1 before phase A so
    # attention's first q-column never waits on its mask.
    mpool = ctx.enter_context(tc.tile_pool(name="mask", bufs=2))
    mask_tiles = {}
    msrc = maskt.rearrange("(kt p) q -> p kt q", p=128)
    def _load_mask(qc):
        m_all = mpool.tile([128, NKT * 512], BF16, tag="m", name="m_all")
        nc.sync.dma_start(m_all.rearrange("p (kt q) -> p kt q", kt=NKT),
                          msrc[:, :, qc * 512:(qc + 1) * 512])
        mask_tiles[qc] = m_all

    # PSUM left stack: pproj(8) -> pv(4) -> st(4)+ot(2); right: py(2).
    ppool = tc.alloc_tile_pool(name="pproj", bufs=2, space="PSUM")

    # ---- Phase A: K projection + first quarter of Q up front --------
    # Q quarters tq=1..3 are deferred into the attention pipeline.
    wpoolq = ctx.enter_context(tc.tile_pool(name="wtsq", bufs=1))
    wq_all = wpoolq.tile([128, NM * C], BF16, tag="wq", name="wq_all")
    with tc.tile_pool(name="wts", bufs=1) as wpool, \
         tc.tile_pool(name="xin", bufs=4) as xpool:
        wk_all = wpool.tile([128, NM * C], BF16, tag="w", name="wk_all")
        for wdst_t, wdram in ((wk_all, wkt), (wq_all, wqt)):
            wsrc = wdram.rearrange("(m p) c -> p m c", p=128)
            wdst = wdst_t.rearrange("p (m c) -> p m c", m=NM)
            nc.sync.dma_start(wdst[:, 0:1], wsrc[:, 0:1])
            nc.sync.dma_start(wdst[:, 1:NM], wsrc[:, 1:NM])
        for pi, (w_all, xdram, dst, boff, ths) in enumerate(
                ((wk_all, kt, kpt_sb, NCT, (0, 1)),
                 (wq_all, qt, qpt_sb, 0, (0,)))):
            for th in ths:               # halves of T
                tcs = (0, 1) if pi == 0 else (0,)
                psq = {}
                for m in range(NM):
                    xm = xpool.tile([128, 1024], BF16, tag="x", name="x")
                    nc.sync.dma_start(
                        xm[:, :], xdram[m * 128:(m + 1) * 128,
                                        th * 1024:(th + 1) * 1024])
                    for ct in range(NCT):
                        lhs = w_all[:, m * C + ct * 128:m * C + (ct + 1) * 128]
                        for tc2 in tcs:
                            if m == 0:
                                psq[(ct, tc2)] = ppool.tile(
                                    [128, 512], F32, tag=f"pp{ct}",
                                    name=f"pp{ct}")
                            nc.tensor.matmul(
                                psq[(ct, tc2)][:, :],
                                lhs, xm[:, tc2 * 512:(tc2 + 1) * 512],
                                start=(m == 0), stop=(m == NM - 1))
                for ct in range(NCT):
                    for tc2 in tcs:
                        tq = th * 2 + tc2
                        nc.vector.tensor_scalar_add(
                            dst[ct][:, tq * 512:(tq + 1) * 512],
                            psq[(ct, tc2)][:, :],
                            bias_sb[:, boff + ct:boff + ct + 1])
    ppool.release()
    _load_mask(0)
    nc.sync.dma_start(wo_all.rearrange("p (c j) -> p c j", c=NCT),
                      wot.rearrange("(c p) j -> p c j", p=128))

    # ---- Phase B: V projection (emitted interleaved into phase C) ---
    pvpool = tc.alloc_tile_pool(name="pv", bufs=1, space="PSUM")
    wpool2 = ctx.enter_context(tc.tile_pool(name="wts2", bufs=1))
    vtpool = ctx.enter_context(tc.tile_pool(name="vtin", bufs=8))
    wv_all = wpool2.tile([128, NM * C], BF16, tag="wv", name="wv")
    nc.sync.dma_start(wv_all.rearrange("p (m c) -> p m c", m=NM),
                      wvt.rearrange("(m p) c -> p m c", p=128))
    vtm_tiles = {}
    xq_tiles = {}

    def emit_q_tail(j):
        # j in 0..11 -> (tq, ct); projects qpt[:, tq-quarter] for tile ct
        tq, ct = 1 + j // NCT, j % NCT
        if ct == 0:
            for m in range(NM):
                xq = vtpool.tile([128, 512], BF16, tag="vt",
                                 name=f"xq{m}")
                nc.sync.dma_start(
                    xq[:, :], qt[m * 128:(m + 1) * 128,
                                 tq * 512:(tq + 1) * 512])
                xq_tiles[m] = xq
        ps = pvpool.tile([128, C], F32, tag="pv", name="pvq")
        for m in range(NM):
            lhs = wq_all[:, m * C + ct * 128:m * C + (ct + 1) * 128]
            nc.tensor.matmul(ps[:, 0:512], lhs, xq_tiles[m][:, :],
                             start=(m == 0), stop=(m == NM - 1))
        nc.vector.tensor_scalar_add(
            qpt_sb[ct][:, tq * 512:(tq + 1) * 512], ps[:, 0:512],
            bias_sb[:, ct:ct + 1])

    def emit_b(tt):
        tg, i = divmod(tt, 4)
        if i == 0:
            for m in range(NM):
                vtm = vtpool.tile([128, 512], BF16, tag="vt",
                                  name=f"vtm{m}")
                nc.sync.dma_start(
                    vtm[:, :], vt[m * 128:(m + 1) * 128,
                                  tg * 512:(tg + 1) * 512])
                vtm_tiles[m] = vtm
        ps = pvpool.tile([128, C], F32, tag="pv", name="pv")
        for m in range(NM):
            nc.tensor.matmul(ps[:, :],
                             vtm_tiles[m][:, i * 128:(i + 1) * 128],
                             wv_all[:, m * C:(m + 1) * C],
                             start=(m == 0), stop=(m == NM - 1))
        dstv = vp_ext[:, tt * VPW:(tt + 1) * VPW].rearrange(
            "p (h e) -> p h e", h=HC)[:, :, 0:DH]
        nc.vector.tensor_copy(
            dstv, ps.rearrange("p (h e) -> p h e", h=HC))

    # ---- Phase C: attention, with phase-D block interleaved per qc --
    stpool = tc.alloc_tile_pool(name="st", bufs=2, space="PSUM", side="right")
    pypool = tc.alloc_tile_pool(name="py", bufs=1, space="PSUM", side="right")
    otpool = tc.alloc_tile_pool(name="ot", bufs=OT_BUFS, space="PSUM", side="right")
    with tc.tile_pool(name="pt", bufs=4) as ptpool, \
         tc.tile_pool(name="ptm", bufs=4) as ptmpool, \
         tc.tile_pool(name="nrm", bufs=2) as nrmpool, \
         tc.tile_pool(name="yev", bufs=1) as ypool:
        NG = NQC * NCT * NKT          # 256 pipeline steps
        ptms = {}                     # g -> masked-prob tile
        ots = {}                      # (qc, hp) -> [ot_h0, ot_h1]
        ye_box = [None]

        def _coords(g):
            qc, r = divmod(g, NCT * NKT)
            hp, ktile = divmod(r, NKT)
            return qc, hp, ktile

        def s1(g):
            # QK^T -> exp -> mask multiply for step g (2 steps ahead of PV)
            qc, hp, ktile = _coords(g)
            if hp == 0 and ktile == 0 and qc + 1 < NQC \
                    and qc + 1 not in mask_tiles:
                _load_mask(qc + 1)
            m_all = mask_tiles[qc]
            st = stpool.tile([128, 1024], F32, tag="st", name="st")
            for h in range(2):
                lhs = kpt_sb[hp][h * 64:(h + 1) * 64,
                                 ktile * 128:(ktile + 1) * 128]
                rhs = qpt_sb[hp][h * 64:(h + 1) * 64,
                                 qc * 512:(qc + 1) * 512]
                nc.tensor.matmul(st[:, h * 512:(h + 1) * 512],
                                 lhs, rhs, start=True, stop=True)
            pt = ptpool.tile([128, 1024], BF16, tag="pt", name="pt")
            nc.scalar.activation(pt[:, :], st[:, :], EXP)
            ptm = ptmpool.tile([128, 1024], BF16, tag="ptm", name="ptm")
            msl = m_all[:, ktile * 512:(ktile + 1) * 512]
            for h in range(2):
                nc.vector.tensor_tensor(
                    ptm[:, h * 512:(h + 1) * 512],
                    pt[:, h * 512:(h + 1) * 512], msl,
                    mybir.AluOpType.mult)
            ptms[g] = ptm

        def s2(g):
            # PV accumulation for step g
            qc, hp, ktile = _coords(g)
            if ktile == 0:
                ots[(qc, hp)] = [otpool.tile([65, 512], F32, tag=f"ot{h}",
                                             name=f"ot{h}")
                                 for h in range(2)]
            ot2 = ots[(qc, hp)]
            ptm = ptms.pop(g)
            for h in range(2):
                hg = hp * 2 + h
                vsl = vp_ext[:, ktile * VPW + hg * 65:
                             ktile * VPW + (hg + 1) * 65]
                nc.tensor.matmul(ot2[h][:, :], vsl,
                                 ptm[:, h * 512:(h + 1) * 512],
                                 start=(ktile == 0),
                                 stop=(ktile == NKT - 1))

        LEAD = 4
        for g0 in range(LEAD):
            s1(g0)
            if g0 < NKT:
                emit_b(g0)
        qtail_sched = {}
        for j in range(12):
            tq = 1 + j // NCT
            qtail_sched[18 + (tq - 1) * 56 + (j % NCT) * 8] = j
        for g in range(NG):
            s2(g)
            if g + LEAD < NG:
                s1(g + LEAD)
                if g + LEAD < NKT:
                    emit_b(g + LEAD)
            if g in qtail_sched:
                emit_q_tail(qtail_sched[g])
            qc, hp, ktile = _coords(g)
            if ktile != NKT - 1:
                continue
            # head-pair epilogue: normalize + evacuate to otn
            ot2 = ots.pop((qc, hp))
            for h in range(2):
                recip = nrmpool.tile([1, 512], F32, tag="rc", name="recip")
                rep = nrmpool.tile([64, 512], F32, tag="rep", name="rep")
                nc.vector.reciprocal(recip[:, :], ot2[h][64:65, :])
                nc.gpsimd.partition_broadcast(rep[:, :], recip[:, :])
                nc.vector.tensor_tensor(
                    otn_sb[hp][h * 64:(h + 1) * 64,
                               qc * 512:(qc + 1) * 512],
                    ot2[h][0:64, :], rep[:, :], mybir.AluOpType.mult)
            # spread previous qcol's output projection across this qcol
            if D_MODE == "interleaved" and qc > 0:
                ye_box[0] = ypool.tile([128, 2 * 512], F32,
                                       tag="ye", name="ye")
                _emit_d_jbs(nc, pypool, ye_box[0], wo_all, otn_sb,
                            qc - 1, range(hp * 2, hp * 2 + 2))
                _emit_d_store(nc, ye_box[0], yt, qc - 1, hp)
            if D_MODE == "interleaved" and qc == NQC - 1 and hp == NCT - 1:
                for pair in range(4):
                    ye = ypool.tile([128, 2 * 512], F32, tag="ye", name="ye")
                    _emit_d_jbs(nc, pypool, ye, wo_all, otn_sb, qc,
                                range(pair * 2, pair * 2 + 2),
                                altpool=pvpool)
                    _emit_d_store(nc, ye, yt, qc, pair)
    otpool.release()
    pypool.release()
    stpool.release()
    pvpool.release()


def kernel(q, k, v, mask, Wq, bq, Wk, bk, Wv, bv, Wo, bo, _trace=False):
    if "nc" not in _CACHED:
        _CACHED["nc"] = _build_nc()
    nc = _CACHED["nc"]

    q = np.asarray(q, np.float32)
    k = np.asarray(k, np.float32)
    v = np.asarray(v, np.float32)
    Wq = np.asarray(Wq, np.float32)
    Wk = np.asarray(Wk, np.float32)
    Wv = np.asarray(Wv, np.float32)
    Wo = np.asarray(Wo, np.float32)
    mask = np.asarray(mask)

    in_maps = []
    for core in range(8):
        b, g = divmod(core, 2)
        csl = slice(g * C, (g + 1) * C)
        im = {
            "qt": np.ascontiguousarray(q[b].T).astype(ml_dtypes.bfloat16),
            "kt": np.ascontiguousarray(k[b].T).astype(ml_dtypes.bfloat16),
            "vt": np.ascontiguousarray(v[b].T).astype(ml_dtypes.bfloat16),
            "wqt": np.ascontiguousarray((Wq[csl, :] / 8.0).T).astype(ml_dtypes.bfloat16),
            "wkt": np.ascontiguousarray(Wk[csl, :].T).astype(ml_dtypes.bfloat16),
            "wvt": np.ascontiguousarray(Wv[csl, :].T).astype(ml_dtypes.bfloat16),
            "wot": np.ascontiguousarray(Wo[:, csl].T).astype(
                ml_dtypes.bfloat16),
            "maskt": np.ascontiguousarray(
                (~mask[b, 0]).T.astype(np.float32)).astype(ml_dtypes.bfloat16),
            "bqt": np.ascontiguousarray(
                (np.asarray(bq, np.float32)[csl] / 8.0).reshape(C, 1)),
            "bkt": np.ascontiguousarray(
                np.asarray(bk, np.float32)[csl].reshape(C, 1)),
        }
        in_maps.append(im)

    res = bass_utils.run_bass_kernel_spmd(
        nc, in_maps, core_ids=list(range(8)), trace=_trace)
    if _trace:
        _CACHED["last_results"] = res
    outs = [r["yt"] for r in res.results]

    y = np.empty((B, T, D), np.float32)
    const = (Wo @ np.asarray(bv, np.float32)
             + np.asarray(bo, np.float32)).astype(np.float32)
    for b in range(B):
        y[b] = (outs[2 * b] + outs[2 * b + 1]).T + const
    return y



# revision 38
# speedup vs baseline: 1.2469x; 1.2469x over previous
"""Trainium2 Bass kernel for masked multi-head attention.

Reference computation (B=4, T=2048, D=1024, H=16, dh=64):
    qp = q @ Wq.T + bq ; kp = k @ Wk.T + bk ; vp = v @ Wv.T + bv
    s  = (qh @ khT) / 8 ; s = where(mask, -1e6, s) ; p = softmax(s)
    o  = p @ vh ; y = o @ Wo.T + bo

Sharding: 8 cores = (batch b in 0..3) x (head-group g in 0..1).
Each core handles batch b and 8 heads (512 channels), computes a partial
y^T (output projection over its 512 channels); host sums core pairs,
transposes, and adds the bias terms.

Per-core device algorithm (v3 — engine-balanced for the TimelineSim cost
model, where matmul cost = out_free_cols only and the ACT exp chain is
the 267us floor; everything else is scheduled around keeping that chain
dense from ~9us on):
  A) Q/K/V projections as fp8(e4m3) DoubleRow matmuls (2 contraction
     m-tiles packed per instruction => 4x fewer PE cycles).  Host supplies
     x/W in fp8 with weights pre-scaled by 8 (keeps them in e4m3 normal
     range); the extra 512x on scores is folded into the exp's scale.
     Only the (ct0, tq0) K/Q slices run before the pipeline; the other
     projection groups are scheduled just-in-time into the step loop, and
     DMAs are ordered by deadline on the (serial) DMA device.
  B) vp stored bf16 in a [t, kt*(8*65)] layout with an 8.0-column per head
     (row-sum channel; 8.0 compensates the 8x-scaled Wv so wot needs no
     rescale).  V-projection groups are emitted once their input quarter
     has landed; the PV stage is allowed to lag (deep ptm pool).
  C) per (qcol, head-pair): ST[k,q] = khT.T @ qhT (f32r) -> exp on ACT
     (scale=1/512, bf16 out) -> mask multiply (one DVE op, mask broadcast
     across the 2 heads) -> PV with q on partitions: for each head/qtile,
     o[q,65] += ptm[k,qtile].T @ [v|8] (N=65 per matmul — half the PE
     cycles of the d-on-partition form).  Normalize with a single DVE
     tensor_tensor (recip broadcast along the head dim), PE-transpose o
     back to [c,q] two steps later (so the PE never waits on the
     normalize), evacuate on GPSIMD.  otn is a 2-deep ring over qcols.
  D) yT[j,t] = sum_c wot[c,j] * otn[c,t]  (bf16) -> DMA out.
"""

import sys
import numpy as np

for _p in ("/opt/trn_rl_repo",):
    if _p not in sys.path:
        sys.path.insert(0, _p)

import ml_dtypes
from contextlib import ExitStack

import concourse.bass as bass
import concourse.tile as tile
from concourse import bacc, mybir
from concourse import bass_utils

B, T, D, H = 4, 2048, 1024, 16
DH = 64          # head dim
HC = 8           # heads per core
C = HC * DH      # 512 channels per core
F32 = mybir.dt.float32
F32R = mybir.dt.float32r
BF16 = mybir.dt.bfloat16
FP8 = mybir.dt.float8e4
EXP = mybir.ActivationFunctionType.Exp
DR = mybir.MatmulPerfMode.DoubleRow

NKT = T // 128       # 16 k-tiles
NQC = 4              # q columns of 512
NCT = C // 128       # 4 channel tiles (= head pairs)
NPAIR = 4            # fp8 contraction m-tile pairs (D=1024 -> 4x(2x128))
VPW = HC * (DH + 1)  # 520
ONES_VAL = 8.0       # compensates 8x-scaled Wv in the row-sum column
EXP_SCALE = 1.0 / 512.0  # (8*Wq)·(8*Wk) => 64x, plus the 1/8 score scale

_CACHED = {}
PRIO_S1 = 100


def _build_nc():
    nc = bacc.Bacc("TRN2", target_bir_lowering=False, debug=False,
                   enable_asserts=False)
    qt8 = nc.dram_tensor("qt8", [128, 2 * NPAIR * 2 * T], FP8,
                         kind="ExternalInput").ap()
    kt8 = nc.dram_tensor("kt8", [128, 2 * NPAIR * 2 * T], FP8,
                         kind="ExternalInput").ap()
    vt8 = nc.dram_tensor("vt8", [128, 2 * NPAIR * 2 * T], FP8,
                         kind="ExternalInput").ap()
    wq8 = nc.dram_tensor("wq8", [128, 2 * NPAIR * 2 * C], FP8,
                         kind="ExternalInput").ap()
    wk8 = nc.dram_tensor("wk8", [128, 2 * NPAIR * 2 * C], FP8,
                         kind="ExternalInput").ap()
    wv8 = nc.dram_tensor("wv8", [128, 2 * NPAIR * 2 * C], FP8,
                         kind="ExternalInput").ap()
    wot = nc.dram_tensor("wot", [C, D], BF16, kind="ExternalInput").ap()
    maskt = nc.dram_tensor("maskt", [T, T], BF16, kind="ExternalInput").ap()
    bqt = nc.dram_tensor("bqt", [C, 1], F32, kind="ExternalInput").ap()
    bkt = nc.dram_tensor("bkt", [C, 1], F32, kind="ExternalInput").ap()
    ident = nc.dram_tensor("ident", [128, 128], BF16,
                           kind="ExternalInput").ap()
    yt = nc.dram_tensor("yt", [D, T], F32, kind="ExternalOutput").ap()

    with tile.TileContext(nc) as tc, ExitStack() as ctx:
        _emit(ctx, tc, qt8, kt8, vt8, wq8, wk8, wv8, wot, maskt, bqt, bkt,
              ident, yt)
    nc.compile()
    return nc


def _emit(ctx, tc, qt8, kt8, vt8, wq8, wk8, wv8, wot, maskt, bqt, bkt,
          ident, yt):
    nc = tc.nc

    # ---- persistent SBUF arrays -------------------------------------
    persist = ctx.enter_context(tc.tile_pool(name="persist", bufs=1))
    # qpt is a 2-deep ring over q-columns (each qc's slice is only read
    # during its own 64 steps; the Q-tail writes qc+1 while qc runs).
    qpt_sb = [persist.tile([128, 2 * 512], F32R, tag=f"qpt{i}",
                           name=f"qpt{i}") for i in range(NCT)]
    kpt_sb = [persist.tile([128, T], F32R, tag=f"kpt{i}", name=f"kpt{i}")
              for i in range(NCT)]
    vp_ext = persist.tile([128, NKT * VPW], BF16, tag="vpext", name="vpext")
    # otn is a ring over qc%2 (the output projection for qc-1 runs while
    # qc's transposes land in the other half).
    otn_sb = [persist.tile([128, 2 * 512], BF16, tag=f"otn{i}",
                           name=f"otn{i}") for i in range(NCT)]
    bias_sb = persist.tile([128, 2 * NCT], F32, tag="bias", name="bias")
    wo_all = persist.tile([128, NCT * D], BF16, tag="wo", name="wo_all")
    id_sb = persist.tile([128, 128], BF16, tag="id", name="id_sb")
    dummy = persist.tile([1, 2], BF16, tag="dm", name="dummy")
    WSZ = NPAIR * 2 * C
    w8_sb = persist.tile([128, 6 * WSZ], FP8, tag="w8", name="w8")

    def _wview(i):
        return w8_sb[:, i * WSZ:(i + 1) * WSZ].rearrange(
            "p (pr s c) -> p pr s c", pr=NPAIR, s=2)

    # (hi, lo) per array
    wq_sb = (_wview(0), _wview(1))
    wk_sb = (_wview(2), _wview(3))
    wv_sb = (_wview(4), _wview(5))

    def _xsrc(t8):
        full = t8.rearrange("p (hl pr s t) -> p hl pr s t", hl=2, pr=NPAIR, s=2)
        return full

    qsrc = _xsrc(qt8)
    ksrc = _xsrc(kt8)
    vsrcq = _xsrc(vt8)

    # Force the ACT exp-table load off the critical path: a dummy exp at
    # t~0 pulls the (1.3us) table load to the very start.
    nc.gpsimd.memset(dummy[:, :], 0.0)
    nc.scalar.activation(dummy[:, :], dummy[:, :], EXP)
    nc.gpsimd.memset(vp_ext[:, :], ONES_VAL)

    # ---- DMA issue order == service order on the (serial) DMA device.
    # Deadline-ordered: bias, K/Q weights+first quarters, first mask
    # chunk, then interleave the rest by first use.
    nc.sync.dma_start(bias_sb[:, 0:NCT],
                      bqt.rearrange("(c p) o -> p (c o)", p=128))
    nc.sync.dma_start(bias_sb[:, NCT:2 * NCT],
                      bkt.rearrange("(c p) o -> p (c o)", p=128))
    mpool = ctx.enter_context(tc.tile_pool(name="mask", bufs=1))
    mask_tiles = {}
    msrc = maskt.rearrange("(kt p) q -> p kt q", p=128)

    def _alloc_mask(qc):
        m_all = mpool.tile([128, NKT * 512], BF16, tag="m", name="m_all")
        mask_tiles[qc] = m_all
        return m_all

    def _load_mask_chunk(qc, c0, n):
        mv = mask_tiles[0].rearrange("p (kt q) -> p kt q", kt=NKT)
        nc.sync.dma_start(mv[:, c0:c0 + n],
                          msrc[:, c0:c0 + n, qc * 512:(qc + 1) * 512])

    def _load_mask(qc):
        _alloc_mask(qc)
        _load_mask_chunk(qc, 0, NKT)

    xbufs = [persist.tile([128, 2 * NPAIR * 2 * 512], FP8, tag=f"xb{i}",
                          name=f"xb{i}").rearrange(
                 "p (hl pr s t) -> p hl pr s t", hl=2, pr=NPAIR, s=2)
             for i in range(4)]
    xa_tiles = {}

    def fetch_x(key, src, tq, buf, part=None):
        # hi|lo quarter into an explicitly-chosen buffer (manual ring: the
        # caller guarantees the previous tenant's readers were emitted).
        xv = xbufs[buf]
        xa_tiles[key] = xv
        parts = (0, 1) if part is None else ((0,) if part == "hi" else (1,))
        for hl in parts:
            nc.sync.dma_start(xv[:, hl],
                              src[:, hl, :, :, tq * 512:(tq + 1) * 512])

    vtpool = ctx.enter_context(tc.tile_pool(name="vtq", bufs=2))
    vtq_tiles = {}

    def _load_v_quarter(vq, part=None):
        if part is None or part == "hi":
            vt = vtpool.tile([128, 2 * NPAIR * 2 * 512], FP8, tag="vtq",
                             name="vtq")
            vtq_tiles[vq] = vt.rearrange("p (hl pr s t) -> p hl pr s t",
                                         hl=2, pr=NPAIR, s=2)
        vv = vtq_tiles[vq]
        parts = (0, 1) if part is None else ((0,) if part == "hi" else (1,))
        for hl in parts:
            nc.sync.dma_start(vv[:, hl],
                              vsrcq[:, hl, :, :, vq * 512:(vq + 1) * 512])

    def _load_w(w_sb, wsrc, hl, ct0=0, ctn=NCT):
        wsv = wsrc.rearrange("p (hl pr s c) -> p hl pr s c", hl=2,
                             pr=NPAIR, s=2)
        nc.sync.dma_start(
            w_sb[hl][:, :, :, ct0 * 128:(ct0 + ctn) * 128],
            wsv[:, hl, :, :, ct0 * 128:(ct0 + ctn) * 128])

    # The DMA device is serial: order transfers by first-use deadline.
    # Full-Wk loads are contiguous (no small-run penalty) and unblock all
    # K channel tiles at once; the Wq remainder + later quarters stream in
    # deadline order behind the exp(0) gate.
    _load_w(wk_sb, wk8, 0)
    fetch_x(("k", 0), ksrc, 0, 0, part="hi")
    _load_w(wq_sb, wq8, 0, 0, 1)
    fetch_x(("q", 0), qsrc, 0, 1, part="hi")
    _load_w(wk_sb, wk8, 1)
    _load_w(wq_sb, wq8, 1, 0, 1)
    fetch_x(("k", 0), ksrc, 0, 0, part="lo")
    fetch_x(("q", 0), qsrc, 0, 1, part="lo")
    _alloc_mask(0)
    _load_mask_chunk(0, 0, 4)
    fetch_x(("k", 1), ksrc, 1, 2)
    _load_w(wv_sb, wv8, 0)
    fetch_x(("k", 2), ksrc, 2, 3)
    _load_mask_chunk(0, 4, 4)
    _load_v_quarter(0, part="hi")
    _load_w(wv_sb, wv8, 1)
    _load_v_quarter(0, part="lo")

    # ---- pools -------------------------------------------------------
    stpool = tc.alloc_tile_pool(name="st", bufs=2, space="PSUM", side="right")
    opool = tc.alloc_tile_pool(name="opsum", bufs=1, space="PSUM")
    scpool = tc.alloc_tile_pool(name="scratch", bufs=2, space="PSUM",
                                side="right")
    with tc.tile_pool(name="pt", bufs=4) as ptpool, \
         tc.tile_pool(name="ptm", bufs=10) as ptmpool, \
         tc.tile_pool(name="nrm", bufs=2) as nrmpool, \
         tc.tile_pool(name="osb", bufs=2) as opool_sb, \
         tc.tile_pool(name="yev", bufs=4) as ypool:
        NG = NQC * NCT * NKT          # 256 pipeline steps
        ptms = {}                     # g -> masked-prob tile
        o_ps_box = {}                 # hp -> psum accumulation tile
        o_sb_box = {}                 # (qc, hp) -> normalized o, q-major

        def _coords(g):
            qc, r = divmod(g, NCT * NKT)
            hp, ktile = divmod(r, NKT)
            return qc, hp, ktile

        TERMS = ((0, 0), (0, 1), (1, 0))  # (w hi/lo, x hi/lo) pairs

        def proj_group(kind, ct, tq):
            # one projection output tile [128c, 512t]: 12 DoubleRow matmuls
            # accumulating Wh·xh + Wh·xl + Wl·xh (fp8 + residual terms).
            w_sb = wk_sb if kind == "k" else wq_sb
            boff = NCT if kind == "k" else 0
            xv = xa_tiles[(kind, tq)]
            ps = scpool.tile([128, 512], F32, tag="sc", name="pp")
            for ti, (whl, xhl) in enumerate(TERMS):
                for pr in range(NPAIR):
                    nc.tensor.matmul(
                        ps[:, :],
                        w_sb[whl][:, pr, :, ct * 128:(ct + 1) * 128],
                        xv[:, xhl, pr],
                        start=(ti == 0 and pr == 0),
                        stop=(ti == len(TERMS) - 1 and pr == NPAIR - 1),
                        perf_mode=DR)
            if kind == "k":
                dst = kpt_sb[ct][:, tq * 512:(tq + 1) * 512]
            else:
                dst = qpt_sb[ct][:, (tq % 2) * 512:(tq % 2 + 1) * 512]
            nc.vector.tensor_scalar_add(
                dst, ps[:, :], bias_sb[:, boff + ct:boff + ct + 1])

        def s1(g):
            # QK^T -> exp -> mask multiply for step g
            qc, hp, ktile = _coords(g)
            m_all = mask_tiles[0]
            with tc.high_priority(offset=PRIO_S1):
                st = stpool.tile([128, 1024], F32, tag="st", name="st")
                for h in range(2):
                    lhs = kpt_sb[hp][h * 64:(h + 1) * 64,
                                     ktile * 128:(ktile + 1) * 128]
                    rhs = qpt_sb[hp][h * 64:(h + 1) * 64,
                                     (qc % 2) * 512:(qc % 2 + 1) * 512]
                    nc.tensor.matmul(st[:, h * 512:(h + 1) * 512],
                                     lhs, rhs, start=True, stop=True)
                pt = ptpool.tile([128, 1024], BF16, tag="pt", name="pt")
                nc.scalar.activation(pt[:, :], st[:, :], EXP, scale=EXP_SCALE)
            ptm = ptmpool.tile([128, 1024], BF16, tag="ptm", name="ptm")
            msl = m_all[:, ktile * 512:(ktile + 1) * 512].rearrange(
                "p (o q) -> p o q", o=1).broadcast_to([128, 2, 512])
            nc.vector.tensor_tensor(
                ptm.rearrange("p (h q) -> p h q", h=2),
                pt.rearrange("p (h q) -> p h q", h=2), msl,
                mybir.AluOpType.mult)
            ptms[g] = ptm

        def s2(g):
            # PV accumulation for step g: q on partitions, N=65 per head
            # and qtile; col 64 of each 65-block is the row-sum channel.
            # PSUM has_written clears at bank granularity on start=True, so
            # only the first group touching each 2KB bank starts; the other
            # groups' first writes land on cleared has_written and overwrite.
            # Layout: col (qt//2)*512 + (qt%2)*130 + h*65 — 4 groups per
            # bank, none crossing the bank boundary.
            qc, hp, ktile = _coords(g)
            if ktile == 0:
                o_ps_box[hp] = opool.tile([128, 1024], F32, tag="o",
                                          name="o_ps")
            o_ps = o_ps_box[hp]
            ptm = ptms.pop(g)
            for qt in range(4):
                for h in range(2):
                    hg = hp * 2 + h
                    vsl = vp_ext[:, ktile * VPW + hg * 65:
                                 ktile * VPW + (hg + 1) * 65]
                    off = (qt // 2) * 512 + (qt % 2) * 130 + h * 65
                    nc.tensor.matmul(
                        o_ps[:, off:off + 65],
                        ptm[:, h * 512 + qt * 128:h * 512 + (qt + 1) * 128],
                        vsl,
                        start=(ktile == 0 and qt % 2 == 0 and h == 0),
                        stop=(ktile == NKT - 1), skip_group_check=True)

        def emit_b(tt):
            # V-projection for t-tile tt: 12 fp8-DR matmuls (tri terms)
            tg, i = divmod(tt, 4)
            vv = vtq_tiles[tg]
            ps = scpool.tile([128, 512], F32, tag="sc", name="pv")
            for ti, (whl, xhl) in enumerate(TERMS):
                for pr in range(NPAIR):
                    nc.tensor.matmul(
                        ps[:, :],
                        vv[:, xhl, pr, :, i * 128:(i + 1) * 128],
                        wv_sb[whl][:, pr],
                        start=(ti == 0 and pr == 0),
                        stop=(ti == len(TERMS) - 1 and pr == NPAIR - 1),
                        perf_mode=DR)
            dstv = vp_ext[:, tt * VPW:(tt + 1) * VPW].rearrange(
                "p (h e) -> p h e", h=HC)[:, :, 0:DH]
            nc.vector.tensor_copy(
                dstv, ps.rearrange("p (h e) -> p h e", h=HC))

        def normalize(qc, hp):
            # recip of the row-sum channels + one TT to scale/narrow o.
            o_ps = o_ps_box.pop(hp)
            opv = o_ps.rearrange("p (b r) -> p b r", b=2)[:, :, 0:260].rearrange(
                "p b (qt h e) -> p b qt h e", qt=2, h=2)
            recip = nrmpool.tile([128, 8], F32, tag="rc", name="recip")
            nc.vector.reciprocal(
                recip.rearrange("p (b qt h) -> p b qt h", b=2, qt=2),
                opv[:, :, :, :, 64])
            o_sb = opool_sb.tile([128, 512], BF16, tag="osb", name="o_sb")
            nc.vector.tensor_tensor(
                o_sb.rearrange("p (b qt h e) -> p b qt h e", b=2, qt=2, h=2),
                opv[:, :, :, :, 0:64],
                recip.rearrange("p (b qt h) -> p b qt h",
                                b=2, qt=2).rearrange(
                    "p b qt h -> p b qt h ()").broadcast_to(
                    [128, 2, 2, 2, 64]),
                mybir.AluOpType.mult)
            o_sb_box[(qc, hp)] = o_sb

        def transpose_o(qc, hp):
            # PE-transpose o [q, c] -> otn [c, q]; evacuate on gpsimd.
            o_sb = o_sb_box.pop((qc, hp))
            for qth in range(2):
                tp = scpool.tile([128, 256], BF16, tag="sc", name="tp")
                for q2 in range(2):
                    qt = qth * 2 + q2
                    nc.tensor.matmul(
                        tp[:, q2 * 128:(q2 + 1) * 128],
                        o_sb[:, qt * 128:(qt + 1) * 128], id_sb[:, :],
                        is_transpose=True, start=True, stop=True)
                nc.vector.tensor_copy(
                    otn_sb[hp][:, (qc % 2) * 512 + qth * 256:
                               (qc % 2) * 512 + (qth + 1) * 256], tp[:, :])

        def emit_d(qc, jbs, tail=False):
            # output projection rows [jb*128] for q-column qc
            ytv = yt.rearrange("(jb p) t -> p jb t", p=128)
            for jb in jbs:
                ye = ypool.tile([128, 512], F32, tag="ye", name="ye")
                if tail:
                    ps = stpool.tile([128, 512], F32, tag="st", name="psy")
                else:
                    ps = scpool.tile([128, 512], F32, tag="sc", name="psy")
                for ct in range(NCT):
                    nc.tensor.matmul(
                        ps[:, :],
                        wo_all[:, ct * D + jb * 128:ct * D + (jb + 1) * 128],
                        otn_sb[ct][:, (qc % 2) * 512:(qc % 2 + 1) * 512],
                        start=(ct == 0), stop=(ct == NCT - 1))
                nc.vector.tensor_copy(ye[:, :], ps[:, :])
                nc.sync.dma_start(
                    ytv[:, jb, qc * 512:(qc + 1) * 512], ye[:, :])

        # ---- schedules ----------------------------------------------
        # projection groups: iter -> list of (kind, ct, tq).  Deadlines:
        # kpt(ct,tq) before s1 step 16ct+4tq (st-slot-paced); qpt(ct,0)
        # before s1 step 16ct; K tq-sweeps free their xa tile early so the
        # 4-slot xa pool can cycle (xa3, then Q-tail quarters).
        proj_sched = {
            0: [("k", 0, 2)],
            1: [("k", 1, 1)],
            2: [("k", 0, 3)],
            3: [("k", 2, 1)],
            4: [("k", 3, 1)],
            5: [("k", 1, 2)],
            6: [("k", 2, 2)],
            7: [("k", 1, 3)],
            8: [("k", 3, 2)],
            9: [("k", 2, 3)],
            10: [("k", 3, 3)],
            11: [("q", 1, 0)],
            12: [("q", 2, 0)],
            13: [("q", 3, 0)],
        }
        # V-proj: emit each t-tile once its vt8 quarter has landed.
        b_sched = {g: [g] for g in range(NKT)}
        # Q-tail quarters tq=1..3 (4 cts each) spread across the pipeline;
        # ct==3 prefetches the next quarter.
        qtail_sched = {}
        for j in range(12):
            tq = 1 + j // NCT
            qtail_sched[48 + (tq - 1) * 56 + (j % NCT) * 8] = j

        # ---- prologue ------------------------------------------------
        proj_group("k", 0, 0)
        proj_group("q", 0, 0)
        for ct in range(1, NCT):
            proj_group("k", ct, 0)
        proj_group("k", 0, 1)
        fetch_x(("k", 3), ksrc, 3, 0)   # reuses xb0 (K tq0 emitted)
        _load_w(wq_sb, wq8, 0, 1, 3)
        _load_mask_chunk(0, 8, 4)
        _load_v_quarter(1)
        _load_w(wq_sb, wq8, 1, 1, 3)
        nc.sync.dma_start(id_sb[:, :], ident)
        _load_mask_chunk(0, 12, 4)
        _load_v_quarter(2)
        _load_v_quarter(3)
        LEAD = 4
        for g0 in range(LEAD):
            s1(g0)
        # ---- the pipeline --------------------------------------------
        for g in range(NG):
            # producers first: emission order IS dependency order for the
            # tile framework, so every projection/V group must precede the
            # s1/s2 that consumes it.
            for kind, ct, tq in proj_sched.get(g, ()):
                proj_group(kind, ct, tq)
            for tt in b_sched.get(g, ()):
                emit_b(tt)
            if g in qtail_sched:
                j = qtail_sched[g]
                tq, ct = 1 + j // NCT, j % NCT
                proj_group("q", ct, tq)
                if ct == NCT - 1 and tq < 3:
                    fetch_x(("q", tq + 1), qsrc, tq + 1, tq + 1)
            if g == 14:
                fetch_x(("q", 1), qsrc, 1, 1)
            if g == 24:
                nc.sync.dma_start(
                    wo_all.rearrange("p (c j) -> p c j", c=NCT),
                    wot.rearrange("(c p) j -> p c j", p=128))
            if g % 64 >= 48 and g % 4 == 0 and g // 64 + 1 < NQC:
                ch = (g % 64 - 48) // 4
                if ch < 4:
                    _load_mask_chunk(g // 64 + 1, ch * 4, 4)
            if g + LEAD < NG:
                s1(g + LEAD)
            s2(g)
            qc, hp, ktile = _coords(g)
            if ktile == NKT - 1:
                normalize(qc, hp)
            elif ktile == 1 and not (qc == 0 and hp == 0):
                pqc, php = (qc, hp - 1) if hp else (qc - 1, NCT - 1)
                transpose_o(pqc, php)
            elif ktile == 4 and qc > 0:
                emit_d(qc - 1, (hp * 2, hp * 2 + 1))
        transpose_o(NQC - 1, NCT - 1)
        for pair in range(4):
            emit_d(NQC - 1, (pair * 2, pair * 2 + 1), tail=True)
    scpool.release()
    opool.release()
    stpool.release()


def _hilo(x):
    # fp8 value + unscaled fp8 residual: x ~ hi + lo to ~0.1% effective
    hi = x.astype(ml_dtypes.float8_e4m3)
    lo = (x - np.asarray(hi, np.float32)).astype(ml_dtypes.float8_e4m3)
    return hi, lo


def _pack_x8(x):
    # [T, D] f32 -> [128, 2*NPAIR*2*T] fp8: [hi || lo] with
    # x8[p, pair, slot, t] = x[t, (2*pair+slot)*128 + p]
    xr = np.ascontiguousarray(
        x.T.reshape(NPAIR, 2, 128, T).transpose(2, 0, 1, 3)
    ).reshape(128, NPAIR * 2 * T)
    hi, lo = _hilo(xr)
    return np.concatenate([hi, lo], axis=1)


def _pack_w8(w):
    # [C, D] f32 (out x in) -> [128, 2*NPAIR*2*C] fp8: [hi || lo] of 8*W
    wr = np.ascontiguousarray(
        (8.0 * w).T.reshape(NPAIR, 2, 128, C).transpose(2, 0, 1, 3)
    ).reshape(128, NPAIR * 2 * C)
    hi, lo = _hilo(wr)
    return np.concatenate([hi, lo], axis=1)


def kernel(q, k, v, mask, Wq, bq, Wk, bk, Wv, bv, Wo, bo, _trace=False):
    if "nc" not in _CACHED:
        _CACHED["nc"] = _build_nc()
    nc = _CACHED["nc"]

    q = np.asarray(q, np.float32)
    k = np.asarray(k, np.float32)
    v = np.asarray(v, np.float32)
    Wq = np.asarray(Wq, np.float32)
    Wk = np.asarray(Wk, np.float32)
    Wv = np.asarray(Wv, np.float32)
    Wo = np.asarray(Wo, np.float32)
    mask = np.asarray(mask)
    ident = np.eye(128, dtype=np.float32).astype(ml_dtypes.bfloat16)

    in_maps = []
    for core in range(8):
        b, g = divmod(core, 2)
        csl = slice(g * C, (g + 1) * C)
        im = {
            "qt8": _pack_x8(q[b]),
            "kt8": _pack_x8(k[b]),
            "vt8": _pack_x8(v[b]),
            "wq8": _pack_w8(Wq[csl, :]),
            "wk8": _pack_w8(Wk[csl, :]),
            "wv8": _pack_w8(Wv[csl, :]),
            "wot": np.ascontiguousarray(Wo[:, csl].T).astype(
                ml_dtypes.bfloat16),
            "maskt": np.ascontiguousarray(
                (~mask[b, 0]).T.astype(np.float32)).astype(ml_dtypes.bfloat16),
            "bqt": np.ascontiguousarray(
                (8.0 * np.asarray(bq, np.float32)[csl]).reshape(C, 1)),
            "bkt": np.ascontiguousarray(
                (8.0 * np.asarray(bk, np.float32)[csl]).reshape(C, 1)),
            "ident": ident,
        }
        in_maps.append(im)

    res = bass_utils.run_bass_kernel_spmd(
        nc, in_maps, core_ids=list(range(8)), trace=_trace)
    if _trace:
        _CACHED["last_results"] = res
    outs = [r["yt"] for r in res.results]

    y = np.empty((B, T, D), np.float32)
    const = (Wo @ np.asarray(bv, np.float32)
             + np.asarray(bo, np.float32)).astype(np.float32)
    for b in range(B):
        y[b] = (outs[2 * b] + outs[2 * b + 1]).T + const
    return y
